# revision 2
# baseline (speedup 1.0000x reference)
"""Causal attention kernel for TRN2, 8 NeuronCores — v2.

Problem: B=4, T=2048, d_in=d_out=1024 fp32 causal attention
    out = softmax(mask(q k^T)/sqrt(d)) @ v,  q/k/v = x @ W{q,k,v}

Sharding (as baseline): 2 cores per batch element; core h of a pair owns
interleaved query tiles {h, h+2, ..., h+14}; identical SPMD stream with
per-core input data (gathered query tensors + additive mask).

Numerics (replaces the baseline's 3-pass fp16 splits):
  The PE multiplies float32r operands at 11 explicit mantissa bits (RNE),
  measured on this hardware. All logit-path operands are pre-rounded (host
  or on-chip Dekker split) to exactly that grid, so the fp32r "main" matmul
  is EXACT; the dropped residuals (<= 2^-12 relative) are restored by two
  fp8e4(e4m3) DoubleRow correction matmuls per stage, with scales chosen so
  corrections either accumulate directly into the main PSUM group (S stage)
  or into a separate PSUM combined by one scaled add (M, A stages).
  Logit-stage cost: 1.0x (main) + 2x0.25 (DR at 0.5c/row, 256-deep) = 1.5x
  of the plain MAC count, vs 3.0x for the baseline's 3-pass fp16.
  Value path (B = P x, out = B Wv) stays 1-pass fp16 (error well under the
  near-argmax amplification floor). End-to-end sim: rel err ~5e-3 vs 2e-2.

Stages per core (MACs in u = 2^20, PE-time in u-equivalents at 1c/row):
  M  = Wq Wk^T          1024u -> 1536u-time (replicated per core)
  A^T= (xq M)^T         1024u -> 1536u-time (direct [i2,q] layout, no transpose)
  S  = A x^T (causal)   1152u -> 1728u-time (corrections join main PSUM)
  B^T= (P x)^T          1152u -> 1152u-time (fp16, direct [i,q] layout)
  out= B Wv             1024u -> 1024u-time (fp16)
  total ~6976u-time ~= 186us PE at 2.4 GHz (vs 11776u = 314us baseline).

Schedule: software-pipelined S/B/out slots as in the baseline, plus: PE
warmup matmuls during the first weight loads (clock ramp + load latency);
the A->S transition emits the main matmuls of slots 0-3 (keys 0:1023 only,
already resident) with their chunk-7 contraction steps deferred past the
last A-combine, hiding the x8T/lx8T/x11Tb/xan/Wv loads that must wait for
the A phase's SBUF space; P transposes stay on the sync queue (a DMA
triggered from the Act queue races with the next Act op rewriting its
source buffer on real hardware).

Measured (CoreSim cost model / axon TRN2): 219658 ns, rel err 5.46e-03
(baseline: 331906 ns, 5.59e-04; gate 2e-2).
"""

import sys
import numpy as np

for _p in (
    "/root/.axon_site",
    "/root/.axon_site/_ro/trn_rl_repo",
    "/root/.axon_site/_ro/pypackages",
    "/opt/trn_rl_repo",
):
    if _p not in sys.path:
        sys.path.append(_p)

import ml_dtypes

B, T, D = 4, 2048, 1024
NQ = 8          # query tile slots per core
NKT = 16        # key tiles per batch
DC = 8          # 128-wide chunks of D
NCORES = 8
DEKKER = 4097.0  # 2^12 + 1: Dekker split at 12 significant bits (m11)

_NC = None


def _build_nc():
    import concourse.tile as tile
    from concourse import bacc, mybir
    from contextlib import ExitStack

    f8 = mybir.dt.float8e4
    f16 = mybir.dt.float16
    f32 = mybir.dt.float32
    f32r = mybir.dt.float32r
    Exp = mybir.ActivationFunctionType.Exp
    Copy = mybir.ActivationFunctionType.Copy
    DR = mybir.MatmulPerfMode.DoubleRow
    AX = mybir.AxisListType.X

    nc = bacc.Bacc("TRN2", target_bir_lowering=False, debug=False)

    def din(name, shape, dt):
        return nc.dram_tensor(name, shape, dt, kind="ExternalInput").ap()

    x11T_d = din("x11T", [D, T], f32r)
    x8T_d = din("x8T", [D, T], f8)
    lx8T_d = din("lx8T", [D, T], f8)
    xan_d = din("xan", [T, D], f16)
    xq11T_d = din("xq11T", [D, NQ * 128], f32r)
    xq8T_d = din("xq8T", [D, NQ * 128], f8)
    lxq8T_d = din("lxq8T", [D, NQ * 128], f8)
    Wq11T_d = din("Wq11T", [D, D], f32r)
    Wk11T_d = din("Wk11T", [D, D], f32r)
    lWq8T_d = din("lWq8T", [D, D], f8)
    Wq8T_d = din("Wq8T", [D, D], f8)
    Wk8T_d = din("Wk8T", [D, D], f8)
    lWk8T_d = din("lWk8T", [D, D], f8)
    Wv_d = din("Wv", [D, D], f16)
    mask_d = din("mask", [128, 256], f32)
    out_d = nc.dram_tensor("out", [NQ, 128, D], f32, kind="ExternalOutput").ap()

    def chunked(ap):  # [D, N] dram -> [128, DC, N] (partition, d-chunk, col)
        return ap.rearrange("(c p) n -> p c n", p=128)

    with tile.TileContext(nc) as tc, ExitStack() as ctx:
        const_pool = ctx.enter_context(tc.tile_pool(name="const", bufs=1))
        mask_sb = const_pool.tile([128, 256], f32)
        nc.scalar.dma_start(out=mask_sb, in_=mask_d)

        with ExitStack() as ma:
            mres = ma.enter_context(tc.tile_pool(name="mres", bufs=1))
            M11 = mres.tile([128, DC, D], f32r)
            lM8 = mres.tile([128, DC, D], f8)
            M8s = mres.tile([128, DC, D], f8)

            xqres = ma.enter_context(tc.tile_pool(name="xqres", bufs=1))
            xq11 = xqres.tile([128, DC, NQ * 128], f32r)
            xq8 = xqres.tile([128, DC, NQ * 128], f8)
            lxq8 = xqres.tile([128, DC, NQ * 128], f8)

            scr = ma.enter_context(tc.tile_pool(name="scr", bufs=2))

            def combine_split(mainps, corrps, corr_scale, m, dst11, dst_l8,
                              dst_s8, l8_scale, s8_scale):
                """fin = main + corr*corr_scale; Dekker-split fin into the
                m11 grid; store fin_hi (f32r), e4m3(lo*l8_scale),
                e4m3(hi*s8_scale)."""
                tcorr = scr.tile([128, 1024], f32, tag="t0", name=f"tc_{m}")
                nc.scalar.activation(out=tcorr, in_=corrps, func=Copy,
                                     scale=corr_scale)
                fin = scr.tile([128, 1024], f32, tag="fin", name=f"fin_{m}")
                nc.vector.tensor_add(fin, mainps, tcorr)
                c1 = scr.tile([128, 1024], f32, tag="c1", name=f"c1_{m}")
                nc.scalar.activation(out=c1, in_=fin, func=Copy, scale=DEKKER)
                c2 = scr.tile([128, 1024], f32, tag="t0", name=f"c2_{m}")
                nc.vector.tensor_sub(c2, c1, fin)
                nc.vector.tensor_sub(dst11, c1, c2)
                lo = scr.tile([128, 1024], f32, tag="c1", name=f"lo_{m}")
                nc.vector.tensor_sub(lo, fin, dst11)
                nc.scalar.activation(out=dst_l8, in_=lo, func=Copy,
                                     scale=l8_scale)
                nc.scalar.activation(out=dst_s8, in_=dst11, func=Copy,
                                     scale=s8_scale)

            # ================= M phase: M = Wq Wk^T =================
            with ExitStack() as mph:
                wqsl = mph.enter_context(tc.tile_pool(name="wqsl", bufs=3))
                wres = mph.enter_context(tc.tile_pool(name="wres", bufs=1))
                Wk11 = wres.tile([128, DC, D], f32r)
                lWq8 = wres.tile([128, DC, D], f8)
                Wq8 = wres.tile([128, DC, D], f8)
                Wk8 = wres.tile([128, DC, D], f8)
                lWk8 = wres.tile([128, DC, D], f8)

                # warmup: keep the PE busy (and its clock ramping) while
                # the first real operands stream in.
                warm = wqsl.tile([128, 512], f32r, tag="warm", name="warm")
                nc.sync.dma_start(out=warm, in_=chunked(x11T_d)[:, 0, 0:512])
                # critical loads: Wk11 chunks split across queues, f8 weights
                # on gpsimd; Wq11 m-slices rotate on sync.
                cwk = chunked(Wk11T_d)
                cwq = chunked(Wq11T_d)
                wq0 = wqsl.tile([128, DC, 128], f32r, tag="wq", name="wq_0")
                nc.sync.dma_start(out=wq0, in_=cwq[:, :, 0:128])
                for c in (0, 4, 6):
                    nc.sync.dma_start(out=Wk11[:, c, :], in_=cwk[:, c, :])
                for c in (1, 3, 5, 7):
                    nc.scalar.dma_start(out=Wk11[:, c, :], in_=cwk[:, c, :])
                nc.gpsimd.dma_start(out=Wk11[:, 2, :], in_=cwk[:, 2, :])
                # f8 weights: first-m slices first so corr(0)/corr(1) start
                # early, then the bulk.
                nc.gpsimd.dma_start(out=Wk8, in_=chunked(Wk8T_d))
                nc.gpsimd.dma_start(out=lWq8[:, :, 0:256],
                                    in_=chunked(lWq8T_d)[:, :, 0:256])
                nc.gpsimd.dma_start(out=lWk8, in_=chunked(lWk8T_d))
                nc.gpsimd.dma_start(out=Wq8[:, :, 0:256],
                                    in_=chunked(Wq8T_d)[:, :, 0:256])
                nc.gpsimd.dma_start(out=lWq8[:, :, 256:1024],
                                    in_=chunked(lWq8T_d)[:, :, 256:1024])
                nc.gpsimd.dma_start(out=Wq8[:, :, 256:1024],
                                    in_=chunked(Wq8T_d)[:, :, 256:1024])
                # prefetch for A phase on gpsimd (after M-critical f8 loads)
                nc.gpsimd.dma_start(out=xq11, in_=chunked(xq11T_d))
                nc.gpsimd.dma_start(out=xq8, in_=chunked(xq8T_d))
                nc.gpsimd.dma_start(out=lxq8, in_=chunked(lxq8T_d))

                mm = mph.enter_context(tc.tile_pool(name="mm", bufs=2,
                                                    space="PSUM"))
                mc = mph.enter_context(tc.tile_pool(name="mc", bufs=2,
                                                    space="PSUM"))
                warmps = mm.tile([128, 512], f32, tag="mm", name="warm_ps")
                for _ in range(6):
                    nc.tensor.matmul(warmps, warm[:, 0:128], warm,
                                     start=True, stop=True)
                for m in range(DC):
                    if m == 0:
                        wq = wq0
                    else:
                        wq = wqsl.tile([128, DC, 128], f32r, tag="wq",
                                       name=f"wq_{m}")
                        nc.sync.dma_start(out=wq,
                                          in_=cwq[:, :, m * 128:(m + 1) * 128])
                    mainps = mm.tile([128, 1024], f32, tag="mm", name=f"mm_{m}")
                    for g in range(2):
                        sl = mainps[:, g * 512:(g + 1) * 512]
                        for c in range(DC):
                            nc.tensor.matmul(
                                sl, wq[:, c, :], Wk11[:, c, g * 512:(g + 1) * 512],
                                start=(c == 0), stop=(c == DC - 1))
                    corrps = mc.tile([128, 1024], f32, tag="mc", name=f"mc_{m}")
                    for g in range(2):
                        sl = corrps[:, g * 512:(g + 1) * 512]
                        for p in range(4):
                            pr = slice(2 * p, 2 * p + 2)
                            gsl = slice(g * 512, (g + 1) * 512)
                            nc.tensor.matmul(
                                sl, lWq8[:, pr, m * 128:(m + 1) * 128],
                                Wk8[:, pr, gsl], perf_mode=DR,
                                start=(p == 0), stop=False)
                            nc.tensor.matmul(
                                sl, Wq8[:, pr, m * 128:(m + 1) * 128],
                                lWk8[:, pr, gsl], perf_mode=DR,
                                start=False, stop=(p == 3))
                    combine_split(mainps, corrps, 2.0 ** -10, m,
                                  M11[:, m, :], lM8[:, m, :], M8s[:, m, :],
                                  2.0 ** 6, 2.0 ** -4)

            # ================= A phase: A^T = (xq M)^T =================
            ares = ctx.enter_context(tc.tile_pool(name="ares", bufs=1, side="right"))
            A11 = ares.tile([128, DC, NQ * 128], f32r)
            lA8 = ares.tile([128, DC, NQ * 128], f8)
            A8s = ares.tile([128, DC, NQ * 128], f8)
            xresA = ctx.enter_context(tc.tile_pool(name="xresA", bufs=1, side="right"))
            x11Ta = xresA.tile([128, DC, 1024], f32r)
            with ExitStack() as aph:
                # prefetch for S phase (keys half 1) while A runs
                nc.sync.dma_start(out=x11Ta,
                                  in_=chunked(x11T_d)[:, :, 0:1024])
                am = aph.enter_context(tc.tile_pool(name="am", bufs=2,
                                                    space="PSUM"))
                ac = aph.enter_context(tc.tile_pool(name="ac", bufs=2,
                                                    space="PSUM"))
                for a in range(DC):
                    asl = slice(a * 128, (a + 1) * 128)
                    mainps = am.tile([128, 1024], f32, tag="am", name=f"am_{a}")
                    for g in range(2):
                        sl = mainps[:, g * 512:(g + 1) * 512]
                        gsl = slice(g * 512, (g + 1) * 512)
                        for c in range(DC):
                            nc.tensor.matmul(
                                sl, M11[:, c, asl], xq11[:, c, gsl],
                                start=(c == 0), stop=(c == DC - 1))
                    corrps = ac.tile([128, 1024], f32, tag="ac", name=f"ac_{a}")
                    for g in range(2):
                        sl = corrps[:, g * 512:(g + 1) * 512]
                        gsl = slice(g * 512, (g + 1) * 512)
                        for p in range(4):
                            pr = slice(2 * p, 2 * p + 2)
                            nc.tensor.matmul(
                                sl, M8s[:, pr, asl], lxq8[:, pr, gsl],
                                perf_mode=DR, start=(p == 0), stop=False)
                            nc.tensor.matmul(
                                sl, lM8[:, pr, asl], xq8[:, pr, gsl],
                                perf_mode=DR, start=False, stop=(p == 3))
                    combine_split(mainps, corrps, 2.0 ** -6, a,
                                  A11[:, a, :], lA8[:, a, :], A8s[:, a, :],
                                  1.0, 2.0 ** -10)
        # ================= S / B / out phases, pipelined per slot ========
        # Slots 0..3 read only keys 0:1023 (x11Ta, already resident), so
        # S(0)/S(1) main matmuls start the moment the A phase drains, hiding
        # the x8T/lx8T/x11Tb/xan/Wv loads that must wait for A's SBUF space.
        xresB = ctx.enter_context(tc.tile_pool(name="xresB", bufs=1))
        x11Tb = xresB.tile([128, DC, 1024], f32r)
        x8T = xresB.tile([128, DC, T], f8)
        lx8T = xresB.tile([128, DC, T], f8)
        vres = ctx.enter_context(tc.tile_pool(name="vres", bufs=1))
        xan = vres.tile([128, NKT, D], f16)
        Wv_sb = vres.tile([128, DC, D], f16)

        cxan = xan_d.rearrange("(kt p) i -> p kt i", p=128)
        for kt in range(2):
            nc.sync.dma_start(out=xan[:, kt, :], in_=cxan[:, kt, :])
        nc.gpsimd.dma_start(out=x8T[:, :, 0:1024],
                            in_=chunked(x8T_d)[:, :, 0:1024])
        nc.sync.dma_start(out=lx8T[:, :, 0:1024],
                          in_=chunked(lx8T_d)[:, :, 0:1024])
        nc.scalar.dma_start(out=x8T[:, :, 1024:2048],
                            in_=chunked(x8T_d)[:, :, 1024:2048])
        for kt in (2, 3):
            nc.gpsimd.dma_start(out=xan[:, kt, :], in_=cxan[:, kt, :])
        nc.gpsimd.dma_start(out=Wv_sb, in_=chunked(Wv_d))

        att = ctx.enter_context(tc.tile_pool(name="att", bufs=1))
        ptp = ctx.enter_context(tc.tile_pool(name="ptp", bufs=1))
        b16p = ctx.enter_context(tc.tile_pool(name="b16p", bufs=1))
        osb = ctx.enter_context(tc.tile_pool(name="osb", bufs=1))
        stat = ctx.enter_context(tc.tile_pool(name="stat", bufs=2))
        rstat = ctx.enter_context(tc.tile_pool(name="rstat", bufs=8))
        sp = ctx.enter_context(tc.tile_pool(name="spsum", bufs=1, space="PSUM"))

        state = [None] * NQ

        def S_main(j, s, off, cs=tuple(range(DC))):
            L = (2 * j + 2) * 128
            jsl = slice(j * 128, (j + 1) * 128)
            for g in range((L + 511) // 512):
                n = min(512, L - g * 512)
                sl = s[:, off + g * 512: off + g * 512 + n]
                xh = x11Ta if g < 2 else x11Tb
                h0 = g * 512 - (0 if g < 2 else 1024)
                for c in cs:
                    nc.tensor.matmul(sl, A11[:, c, jsl], xh[:, c, h0:h0 + n],
                                     start=(c == 0), stop=False)

        def S_corr(j, s, off):
            L = (2 * j + 2) * 128
            jsl = slice(j * 128, (j + 1) * 128)
            for g in range((L + 511) // 512):
                n = min(512, L - g * 512)
                gsl = slice(g * 512, g * 512 + n)
                sl = s[:, off + g * 512: off + g * 512 + n]
                for p in range(4):
                    pr = slice(2 * p, 2 * p + 2)
                    nc.tensor.matmul(sl, lA8[:, pr, jsl], x8T[:, pr, gsl],
                                     perf_mode=DR, start=False, stop=False)
                    nc.tensor.matmul(sl, A8s[:, pr, jsl], lx8T[:, pr, gsl],
                                     perf_mode=DR, start=False,
                                     stop=(p == 3))

        def S_smax(j, s, off, tq=None):
            nk = 2 * j + 2
            L = nk * 128
            sl = s[:, off: off + L]
            nc.vector.tensor_add(s[:, off + L - 256: off + L],
                                 s[:, off + L - 256: off + L], mask_sb)
            nmx = stat.tile([128, 1], f32, tag="nmx", name=f"nmx_{j}")
            nc.vector.reduce_max(nmx, sl, axis=AX, negate=True)
            nbias = stat.tile([128, 1], f32, tag="nbias", name=f"nb_{j}")
            nc.vector.tensor_scalar_mul(nbias, nmx, 0.03125)
            P = att.tile([128, 2048], f16, tag="P", name=f"p_{j}")
            rsum = stat.tile([128, 1], f32, tag="rsum", name=f"rs_{j}")
            nc.scalar.activation(out=P[:, :L], in_=sl, func=Exp,
                                 bias=nbias, scale=0.03125, accum_out=rsum)
            rinv = rstat.tile([128, 1], f32, tag="rinv", name=f"ri_{j}")
            nc.vector.reciprocal(rinv, rsum)
            PT = ptp.tile([128, NKT, 128], f16, tag="PT", name=f"pt_{j}")
            (tq or nc.sync).dma_start_transpose(PT[:, :nk, :], P[:, :L])
            for kt in (2 * j + 4, 2 * j + 5):
                if kt < NKT:
                    nc.gpsimd.dma_start(out=xan[:, kt, :], in_=cxan[:, kt, :])
            state[j] = (PT, rinv)

        def emit_S(j):
            s = sp.tile([128, 2048], f32, tag="S", name=f"s_{j}")
            S_main(j, s, 0)
            S_corr(j, s, 0)
            S_smax(j, s, 0)

        def emit_BT(j):
            nk = 2 * j + 2
            PT, rinv = state[j]
            bt = btp.tile([128, DC, 128], f32, tag="bt", name=f"bt_{j}")
            for c in range(DC):
                csl = slice(c * 128, (c + 1) * 128)
                for kc in range(nk):
                    nc.tensor.matmul(
                        bt[:, c, :], xan[:, kc, csl], PT[:, kc, :],
                        start=(kc == 0), stop=(kc == nk - 1))
            B16 = b16p.tile([128, DC, 128], f16, tag="B16", name=f"b16_{j}")
            nc.vector.tensor_copy(B16, bt)
            state[j] = (B16, rinv)

        def emit_out(j):
            B16, rinv = state[j]
            ops = op.tile([128, 1024], f32, tag="op", name=f"op_{j}")
            for g in range(2):
                sl = ops[:, g * 512:(g + 1) * 512]
                gsl = slice(g * 512, (g + 1) * 512)
                for c in range(DC):
                    nc.tensor.matmul(
                        sl, B16[:, c, :], Wv_sb[:, c, gsl],
                        start=(c == 0), stop=(c == DC - 1))
            out_sb = osb.tile([128, 1024], f32, tag="osb", name=f"osb_{j}")
            nc.scalar.activation(out=out_sb, in_=ops, func=Copy, scale=rinv)
            nc.gpsimd.dma_start(out=out_d[j], in_=out_sb)
            state[j] = None

        # Transition: emit all main matmuls for slots 0-3 (keys 0:1023,
        # x11Ta-resident) before any correction, so the PE stays busy while
        # x8T/lx8T stream into the space the A phase just freed. S2 and S3
        # share one sp tile (banks 0-1 / 2-3) so only the small s01 pool's
        # release gates the opsum allocation (avoiding release cycles).
        btp = ctx.enter_context(tc.tile_pool(name="btpsum", bufs=1,
                                             space="PSUM"))
        with ExitStack() as s01ctx:
            sp01 = s01ctx.enter_context(
                tc.tile_pool(name="sp01", bufs=1, space="PSUM", side="right"))
            s01 = sp01.tile([128, 1024], f32, tag="s01")
            s23 = sp.tile([128, 2048], f32, tag="S", name="s_23")
            head = tuple(range(DC - 1))
            S_main(0, s01, 0, cs=head)
            S_main(1, s01, 512, cs=head)
            S_main(2, s23, 0, cs=head)
            S_main(3, s23, 1024, cs=head)
            for j_, st_, of_ in ((0, s01, 0), (1, s01, 512), (2, s23, 0),
                                 (3, s23, 1024)):
                S_main(j_, st_, of_, cs=(DC - 1,))
            S_corr(0, s01, 0)
            S_smax(0, s01, 0)
            S_corr(1, s01, 512)
            S_smax(1, s01, 512)
            S_corr(2, s23, 0)
            S_smax(2, s23, 0)
            S_corr(3, s23, 1024)
            S_smax(3, s23, 1024)
        for q in range(4):
            nc.sync.dma_start(
                out=x11Tb[:, :, q * 256:(q + 1) * 256],
                in_=chunked(x11T_d)[:, :, 1024 + q * 256: 1280 + q * 256])
        nc.sync.dma_start(out=lx8T[:, :, 1024:2048],
                          in_=chunked(lx8T_d)[:, :, 1024:2048])
        op = ctx.enter_context(tc.tile_pool(name="opsum", bufs=1, space="PSUM"))
        emit_BT(0)
        emit_BT(1)
        emit_out(0)
        emit_BT(2)
        emit_out(1)
        for j in range(4, NQ):
            emit_S(j)
            emit_BT(j - 1)
            emit_out(j - 2)
        emit_BT(NQ - 1)
        emit_out(NQ - 2)
        emit_out(NQ - 1)

    nc.compile()
    return nc


def _get_nc():
    global _NC
    if _NC is None:
        _NC = _build_nc()
    return _NC


def _rne11(v64):
    """Round fp64 values to 12 significant bits (11 explicit), RNE —
    the grid the PE's float32r datapath multiplies on."""
    m, e = np.frexp(v64)
    return np.ldexp(np.round(m * 4096.0) / 4096.0, e)


def _prep_inputs(vector, W_queries, W_keys, W_values):
    F8 = ml_dtypes.float8_e4m3
    x64 = np.asarray(vector, dtype=np.float32).astype(np.float64)
    Wq64 = np.asarray(W_queries, dtype=np.float32).astype(np.float64)
    Wk64 = np.asarray(W_keys, dtype=np.float32).astype(np.float64)
    Wv = np.asarray(W_values, dtype=np.float32)

    x11 = _rne11(x64)                      # [B, T, D]
    lx = x64 - x11
    x11T = np.ascontiguousarray(x11.transpose(0, 2, 1)).astype(np.float32)
    x8T = x11T.astype(F8)
    lx8T = np.ascontiguousarray((lx * 2.0 ** 10).transpose(0, 2, 1)).astype(F8)
    xan = x64.astype(np.float16)

    WqT64 = Wq64.T
    WkT64 = Wk64.T
    Wq11T = _rne11(WqT64)
    Wk11T = _rne11(WkT64)
    lWq8T = ((WqT64 - Wq11T) * 2.0 ** 10).astype(F8)
    lWk8T = ((WkT64 - Wk11T) * 2.0 ** 10).astype(F8)
    Wq11T32 = np.ascontiguousarray(Wq11T.astype(np.float32))
    Wk11T32 = np.ascontiguousarray(Wk11T.astype(np.float32))
    Wq8T = Wq11T32.astype(F8)
    Wk8T = Wk11T32.astype(F8)
    Wv16 = Wv.astype(np.float16)

    r = np.arange(128)[:, None]
    c2 = np.arange(256)[None, :]
    masks = [
        np.where(c2 <= h * 128 + r, np.float32(0.0),
                 np.float32(-1e30)).astype(np.float32)
        for h in (0, 1)
    ]

    in_maps = []
    for core in range(NCORES):
        b, h = core // 2, core % 2

        def gather(full):  # [D, T] -> [D, NQ*128] query-tile gather
            return np.ascontiguousarray(
                full.reshape(D, NKT, 128)[:, h::2, :].reshape(D, NQ * 128))

        in_maps.append({
            "x11T": x11T[b], "x8T": x8T[b], "lx8T": lx8T[b], "xan": xan[b],
            "xq11T": gather(x11T[b]), "xq8T": gather(x8T[b]),
            "lxq8T": gather(lx8T[b]),
            "Wq11T": Wq11T32, "Wk11T": Wk11T32,
            "lWq8T": lWq8T, "Wq8T": Wq8T, "Wk8T": Wk8T, "lWk8T": lWk8T,
            "Wv": Wv16, "mask": masks[h],
        })
    return in_maps


def kernel(vector, W_queries, W_keys, W_values):
    from concourse.bass_utils import run_bass_kernel_spmd

    in_maps = _prep_inputs(vector, W_queries, W_keys, W_values)
    res = run_bass_kernel_spmd(_get_nc(), in_maps, core_ids=list(range(NCORES)))
    out = np.empty((B, T, D), dtype=np.float32)
    for core in range(NCORES):
        b, h = core // 2, core % 2
        o = res.results[core]["out"]
        for j in range(NQ):
            t = 2 * j + h
            out[b, t * 128:(t + 1) * 128, :] = o[j]
    return out
